# revision 26
# baseline (speedup 1.0000x reference)
"""Channel-attention (CAM) kernel for Trainium2, 8 NeuronCores.

Reference computation (per batch b):
    A   = x[b].reshape(L, C)            # L = 48^3 = 110592, C = 256
    G   = A^T A                          # [C, C] Gram matrix
    S   = softmax(G, axis=-1)
    out = gamma * (A @ S) + x[b]

Algebraic fold: out = A @ (gamma*S + I) since A @ I == x.  This removes
the residual add AND the second read of x: HBM traffic is the floor
(read 28.3 MB + write 28.3 MB per core).  A^T (bf16) stays resident in
SBUF between the phases.

v3 schedule (measured findings from the v0/v2 traces):
  * A^T is built with `is_transpose` PE transposes (~65 ns per 128x128
    tile, half the cost of the identity-matmul transposes) writing
    bf16 PSUM, drained in 3-tile batches by the scalar engine.  fp8 /
    DoubleRow was tried and abandoned: on HW a DoubleRow matmul streams
    columns at the same rate as bf16, so fp8 buys no time here.
  * One AllReduce per Gram (128 KB bf16): AR0 after batch-0's reads
    (~55 us), AR1 after batch-1's.  AR1's trigger is gated on AR0's
    completion (a globally synced event) so its entry barrier doesn't
    re-pay rank start-skew.  The runtime's one-time rank barrier
    (~25-60 us) overlaps the read stream and only floors AR0's start.
  * Reads alternate the sync/vector DMA queues and stores alternate
    the sync/scalar queues (one HW queue tops out ~260-320 GB/s; two
    get closer to the 358 GB/s core limit).  xf/ot pools are 4 deep so
    buffer recycling never gates the streams.
  * Engine budget per supertile (cadence target = DMA ~4.6 us):
    tensor: gram 24x131ns + 24 transposes x65ns = 4.7 us   [phase 1]
            Y 24x131ns = 3.1 us                            [phase 2]
    vector: two f32->bf16 casts = 1.9 us + 1 read issue    [phase 1]
            3 of 6 Y-PSUM drains = 2.1 us                  [phase 2]
    scalar: 4 A^T drains = 3.6 us                          [phase 1]
            3 Y drains + 1 store issue                     [phase 2]
    gpsimd: collective staging + triggers only
"""

import numpy as np
from contextlib import ExitStack

import concourse.bass as bass
import concourse.tile as tile
from concourse import bacc, mybir
from concourse.bass import ts
from concourse.bass_utils import run_bass_kernel_spmd
from concourse.masks import make_identity

F32 = mybir.dt.float32
BF16 = mybir.dt.bfloat16
FP16 = mybir.dt.float16
AF = mybir.ActivationFunctionType
# Gram entries reach ~L (110592) > fp16 max; staged as G/4 (max ~27.6k)
# and rescaled exactly inside the softmax (exp(4*g4 + 4*nmx)).  fp8 was
# tried for the AR payload and produced NaN (e4m3 overflow semantics in
# the reduce tree) - fp16 is the floor here.
AR_SCALE = 0.25

N_CORES = 8
B = 2
L = 48 * 48 * 48          # 110592
C = 256
L_SH = L // N_CORES       # 13824 rows per core per batch
ROWS = B * L_SH           # 27648 rows per core
P = 128
RPP = 12                  # rows per partition per supertile
HPP = RPP // 2            # rows per partition per half-supertile
SROWS = P * RPP           # 1536 rows per supertile
HROWS = P * HPP           # 768 rows per half-supertile
SPB = L_SH // SROWS       # 9 supertiles per batch
S_TOT = B * SPB           # 18 supertiles per core

_CACHE: dict = {}


def _build():
    nc = bacc.Bacc(
        "TRN2", target_bir_lowering=False, debug=False, num_devices=N_CORES
    )
    x_dram = nc.dram_tensor("x", [ROWS, C], F32, kind="ExternalInput")
    g_dram = nc.dram_tensor("gamma", [1, 1], F32, kind="ExternalInput")
    o_dram = nc.dram_tensor("out", [ROWS, C], F32, kind="ExternalOutput")
    cc_in = [
        nc.dram_tensor(f"cc_in{g}", [2 * P, C], FP16, kind="Internal")
        for g in range(B)
    ]
    cc_out = [
        nc.dram_tensor(
            f"cc_out{g}", [2 * P, C], FP16, kind="Internal",
            addr_space="Shared",
        )
        for g in range(B)
    ]
    X, GAM, OUT = x_dram.ap(), g_dram.ap(), o_dram.ap()
    GROUPS = [list(range(N_CORES))]

    def x_half(s, h):
        # partition p holds rows s*SROWS + h*HROWS + p*HPP + (0..HPP-1)
        r0 = s * SROWS + h * HROWS
        return X[r0 : r0 + HROWS, :].rearrange("(p j) c -> p j c", j=HPP)

    def o_half(s, h):
        r0 = s * SROWS + h * HROWS
        return OUT[r0 : r0 + HROWS, :].rearrange("(p j) c -> p j c", j=HPP)

    with tile.TileContext(nc) as tc, ExitStack() as octx:
        constp = octx.enter_context(tc.tile_pool(name="const", bufs=1))
        identb = constp.tile([P, P], BF16, name="identb", tag="identb")
        make_identity(nc, identb[:])
        gam_sb = constp.tile([1, 1], F32, name="gam_sb", tag="gam_sb")
        gam_bc = constp.tile([P, 1], F32, name="gam_bc", tag="gam_bc")
        # m_bf[2b+q] = gamma * softmax(G_b)[qP:(q+1)P, :] + I-block
        m_bf = [
            constp.tile([P, C], BF16, name=f"mbf{i}", tag=f"mbf{i}")
            for i in range(4)
        ]

        atp = octx.enter_context(tc.tile_pool(name="at", bufs=S_TOT))
        xbp = octx.enter_context(tc.tile_pool(name="xb", bufs=4))
        xfp = octx.enter_context(tc.tile_pool(name="xf", bufs=8))
        otp = octx.enter_context(tc.tile_pool(name="ot", bufs=3))
        gsp = octx.enter_context(tc.tile_pool(name="gs", bufs=2))
        smp = octx.enter_context(tc.tile_pool(name="smx", bufs=1))
        psg = octx.enter_context(tc.tile_pool(name="psg", bufs=1, space="PSUM"))
        pst = octx.enter_context(tc.tile_pool(name="pst", bufs=3, space="PSUM"))
        psy = octx.enter_context(tc.tile_pool(name="psy", bufs=3, space="PSUM"))

        g_ps = [
            psg.tile([P, 2, C], F32, name=f"g{b}", tag=f"g{b}")
            for b in range(B)
        ]
        ats: dict = {}

        xbs: dict = {}

        def load_cast(s):
            xb = xbp.tile([P, RPP, C], BF16, name="xb", tag="xb")
            xbs[s] = xb
            for h in range(2):
                xf = xfp.tile([P, HPP, C], F32, name="xf", tag="xf")
                # reads split across both HW queues; the scalar-queue
                # read is emitted before this iteration's A^T drains so
                # it is never compute-gated, and 8 xf bufs keep 4
                # transfers outstanding per queue
                eng = nc.sync if h == 0 else nc.scalar
                eng.dma_start(xf[:], x_half(s, h))
                nc.vector.tensor_copy(xb[:, ts(h, HPP), :], xf[:])
            return xb

        def gram(s, xb):
            b = s // SPB
            s_in_b = s % SPB
            for j in range(RPP):
                first = s_in_b == 0 and j == 0
                last = s_in_b == SPB - 1 and j == RPP - 1
                for m in range(2):
                    nc.tensor.matmul(
                        g_ps[b][:, m, :], xb[:, j, ts(m, P)], xb[:, j, :],
                        start=first, stop=last,
                    )

        def tp_work(s):
            xb = xbs[s]
            at = atp.tile([P, RPP, 2, P], BF16, name="at", tag="at")
            ats[s] = at
            for q4 in range(4):
                tpp = pst.tile([P, 3, 2, P], BF16, name="tpp", tag="tpp")
                for jj in range(3):
                    j = 3 * q4 + jj
                    for blk in range(2):
                        nc.tensor.transpose(
                            tpp[:, jj, blk, :],
                            xb[:, j, ts(blk, P)],
                            identb[:],
                        )
                nc.scalar.copy(at[:, 3 * q4 : 3 * q4 + 3, :, :], tpp[:])

        def stage_and_ar(g):
            # staging on scalar: its queue is far ahead of vector's here
            gsb = gsp.tile([P, 2, C], FP16, name="gsb", tag="gsb")
            nc.scalar.mul(gsb[:], g_ps[g][:], AR_SCALE)
            for m in range(2):
                nc.gpsimd.dma_start(cc_in[g].ap()[ts(m, P), :], gsb[:, m, :])
            nc.gpsimd.collective_compute(
                "AllReduce",
                mybir.AluOpType.add,
                replica_groups=GROUPS,
                ins=[cc_in[g].ap()[:, :]],
                outs=[cc_out[g].ap()[:, :]],
            )

        def softmax(b):
            gf = smp.tile([P, 2, C], FP16, name=f"gf{b}", tag=f"gf{b}")
            for m in range(2):
                # b1's loads ride sync so a not-yet-done AR1 can't stall
                # the scalar drain/store stream mid-ywork
                eng = nc.scalar if b == 0 else nc.sync
                eng.dma_start(gf[:, m, :], cc_out[b].ap()[ts(m, P), :])
            for m in range(2):
                i = 2 * b + m
                nmx = smp.tile([P, 1], F32, name="nmx", tag="nmx")
                nc.vector.tensor_reduce(
                    nmx[:], gf[:, m, :],
                    axis=mybir.AxisListType.X,
                    op=mybir.AluOpType.max,
                    negate=True,
                )
                # undo AR_SCALE exactly: exp((g - max)) = exp(4*g4 + 4*nmx)
                nmx4 = smp.tile([P, 1], F32, name="nmx4", tag="nmx4")
                nc.vector.tensor_scalar_mul(nmx4[:], nmx[:], 1.0 / AR_SCALE)
                ex = smp.tile([P, C], F32, name="ex", tag="ex")
                ssum = smp.tile([P, 1], F32, name="ssum", tag="ssum")
                nc.scalar.activation(
                    ex[:], gf[:, m, :], AF.Exp, bias=nmx4[:],
                    scale=1.0 / AR_SCALE,
                    accum_out=ssum[:],
                )
                inv = smp.tile([P, 1], F32, name="inv", tag="inv")
                nc.vector.reciprocal(inv[:], ssum[:])
                sc = smp.tile([P, 1], F32, name="sc", tag="sc")
                nc.vector.tensor_mul(sc[:], inv[:], gam_bc[:])
                nc.scalar.activation(m_bf[i][:], ex[:], AF.Copy, scale=sc[:])
                # fold the residual: M = gamma*S + I (diagonal block m)
                nc.vector.tensor_add(
                    m_bf[i][:, ts(m, P)], m_bf[i][:, ts(m, P)], identb[:]
                )

        def ywork(s):
            b, at = s // SPB, ats[s]
            for h in range(2):
                ot = otp.tile([P, HPP, C], F32, name="ot", tag="ot")
                for jj3 in range(HPP // 2):
                    y = psy.tile([P, 2, C], F32, name="y", tag="y")
                    for q in range(2):
                        j = h * HPP + 2 * jj3 + q
                        nc.tensor.matmul(
                            y[:, q, :], at[:, j, 0, :], m_bf[2 * b][:],
                            start=True, stop=False,
                        )
                        nc.tensor.matmul(
                            y[:, q, :], at[:, j, 1, :], m_bf[2 * b + 1][:],
                            start=False, stop=True,
                        )
                    dst = ot[:, ts(jj3, 2), :]
                    if jj3 == 1:
                        nc.scalar.activation(dst, y[:], AF.Copy)
                    else:
                        nc.vector.tensor_copy(dst, y[:])
                # stores rotate HW DMA queues (sync is free in phase 2;
                # gpsimd frees up after AR1 completes, so b1 gets 3 queues)
                if b == 1:
                    eng = [nc.scalar, nc.sync, nc.gpsimd][(2 * s + h) % 3]
                else:
                    eng = nc.scalar if h == 0 else nc.sync
                eng.dma_start(o_half(s, h), ot[:])

        # ---------------- phase 1 ----------------
        # transposes run TP_LAG supertiles behind the grams so the
        # AR-gating gram(8)/gram(17) stop-matmuls finish read-bound,
        # not tensor-queue-bound
        TP_LAG = 3
        TP_TAIL = 4   # last 4 supertiles' transposes run in the b0 window
        for s in range(S_TOT):
            xb = load_cast(s)
            gram(s, xb)
            if s == SPB - 1:
                # gamma load deferred here so it doesn't gate the
                # collective bootstrap (rank-barrier entry) at t~0
                nc.scalar.dma_start(gam_sb[:], GAM[:, :])
                nc.gpsimd.partition_broadcast(gam_bc[:], gam_sb[:])
                stage_and_ar(0)
            if s == S_TOT - 1:
                # gate AR1's trigger on AR0 COMPLETION (globally synced)
                # so its entry barrier doesn't re-pay the start skew
                ccw = gsp.tile([1, C], BF16, name="ccw", tag="ccw")
                nc.gpsimd.dma_start(ccw[:], cc_out[0].ap()[0:1, :])
                stage_and_ar(1)
            if TP_LAG <= s < S_TOT - TP_TAIL + TP_LAG:
                tp_work(s - TP_LAG)

        # ---------------- phase 2 ----------------
        # softmax(1) is emitted two supertiles before batch-0's ywork
        # ends: the engines stay busy on s7/s8 while AR1 lands, so the
        # b0->b1 boundary has no pipeline-refill bubble
        softmax(0)
        for s in range(SPB - 2):
            ywork(s)
            # deferred tail transposes ride the b0 window's tensor slack
            # (back half, so the store stream ramps unimpeded); their xb
            # tiles are exactly the ones still live in the pool
            if s >= SPB - 2 - TP_TAIL:
                tp_work(S_TOT - TP_TAIL + s - (SPB - 2 - TP_TAIL))
        softmax(1)
        for s in range(SPB - 2, S_TOT):
            ywork(s)

    nc.compile()
    return nc


def _get_nc():
    if "nc" not in _CACHE:
        _CACHE["nc"] = _build()
    return _CACHE["nc"]


def kernel(x: np.ndarray, gamma: np.ndarray, **_kw) -> np.ndarray:
    nc = _get_nc()
    x = np.asarray(x, dtype=np.float32)
    orig_shape = x.shape
    x3 = x.reshape(B, L, C)
    gam = np.asarray(gamma, dtype=np.float32).reshape(1, 1)
    in_maps = []
    for k in range(N_CORES):
        shard = np.ascontiguousarray(
            x3[:, k * L_SH : (k + 1) * L_SH, :]
        ).reshape(ROWS, C)
        in_maps.append({"x": shard, "gamma": gam})
    res = run_bass_kernel_spmd(nc, in_maps, core_ids=list(range(N_CORES)))
    out = np.empty((B, L, C), dtype=np.float32)
    for k in range(N_CORES):
        out[:, k * L_SH : (k + 1) * L_SH, :] = res.results[k]["out"].reshape(
            B, L_SH, C
        )
    return out.reshape(orig_shape)
